# revision 28
# baseline (speedup 1.0000x reference)
"""Decoder block (single-head causal attention + GELU FFN) on 8 TRN2 NeuronCores.

Sharding: pure data parallel, no collectives. Core c handles batch b = c//2 and
1024 query tokens of that batch, chosen as four 256-token chunks that balance
the causal-attention workload:
  even cores (half 0): chunks 0, 3, 4, 7
  odd  cores (half 1): chunks 1, 2, 5, 6
The slot pairing makes the static k-tile counts per slot (4, 8, 12, 16) cover
both cores' needs with minimal waste (ideal is (2..16); the gap is zeroed by
the data-driven qpos mask). The SPMD program is identical on every core; all
per-core differences are data (which tokens are in xq, qpos values that drive
on-chip causal-mask creation).

Performance structure:
  - every matmul operand is fp16 (same PE rate as fp32r, half the DMA/SBUF)
  - K and V projections fused into one pass over x (x read once)
  - K^T and V stay resident in SBUF (no DRAM round-trips)
  - DMA issue split across engines: weights on gpsimd, x on sync, stores on
    scalar, consts on scalar — so tile back-pressure in one stream never
    delays another
  - all long-lived tiles (incl. wq/wo/xq) live in one pool so their loads are
    not gated on earlier phases' SBUF frees
  - scalar engine runs only Identity in P1 and only Exp in P2 (activation
    table reloads cost 1.3us each)
"""

import numpy as np

D = 1024  # model dim
S = 2048  # sequence length
B = 4  # batch
M = 4096  # FFN dim
CH = 256  # q chunk (slot) size
NSLOT = 4  # q slots per core
NDT = D // 128  # 8 d-tiles
N_CORES = 8
NKT = [4, 8, 12, 16]  # k-tiles per slot (static max over the two paired cores)

_PROGRAM = None  # cached compiled program


def _build_program():
    import sys

    if "/opt/trn_rl_repo" not in sys.path:
        sys.path.insert(0, "/opt/trn_rl_repo")
    import concourse.bass as bass
    import concourse.tile as tile
    import concourse.mybir as mybir
    from concourse import bacc
    from concourse.bass import ts

    dt = mybir.dt
    AF = mybir.ActivationFunctionType
    ALU = mybir.AluOpType
    F32, F32R, F16 = dt.float32, dt.float32r, dt.float16

    nc = bacc.Bacc("TRN2", target_bir_lowering=False, debug=False, num_devices=8)

    # ---------------- DRAM I/O ----------------
    xT = nc.dram_tensor("xT", [D, S], F16, kind="ExternalInput").ap()
    xkT = nc.dram_tensor("xkT", [D, S // 2], F16, kind="ExternalInput").ap()
    xoT = nc.dram_tensor("xoT", [D, 4 * CH], F16, kind="ExternalInput").ap()
    wqT = nc.dram_tensor("wqT", [D, D], F16, kind="ExternalInput").ap()
    wkT = nc.dram_tensor("wkT", [D, D], F16, kind="ExternalInput").ap()
    wvT = nc.dram_tensor("wvT", [D, D], F16, kind="ExternalInput").ap()
    woT = nc.dram_tensor("woT", [D, D], F16, kind="ExternalInput").ap()
    wfT = nc.dram_tensor("wfT", [D, M], F16, kind="ExternalInput").ap()
    bq = nc.dram_tensor("bq", [128, D // 128], F32, kind="ExternalInput").ap()
    bk = nc.dram_tensor("bk", [128, D // 128], F32, kind="ExternalInput").ap()
    bo2 = nc.dram_tensor("bo2", [128, D // 128], F32, kind="ExternalInput").ap()
    bfT = nc.dram_tensor("bfT", [128, M // 128], F32, kind="ExternalInput").ap()
    qpos = nc.dram_tensor("qpos", [1, 4 * CH], F32R, kind="ExternalInput").ap()
    iota_kt = nc.dram_tensor("iota_kt", [128, S // 128], F32, kind="ExternalInput").ap()
    ffT = nc.dram_tensor("ffT", [M, 4 * CH], F16, kind="ExternalOutput").ap()

    with tile.TileContext(nc) as tc:
        with (
            tc.tile_pool(name="const", bufs=1) as cpool,
            tc.tile_pool(name="psum", bufs=1, space="PSUM") as pspool,
        ):
            # ---------------- constants (scalar engine issues these) --------
            ones_col_bf = cpool.tile([128, 1], F16, name="ones_col_bf", tag="ones_col_bf")
            nc.vector.memset(ones_col_bf[:], 1.0)
            ones_row_f = cpool.tile([1, 128], F32, name="ones_row_f", tag="ones_row_f")
            nc.vector.memset(ones_row_f[:], 1.0)
            ones_row = cpool.tile([1, 128], F32R, name="ones_row", tag="ones_row")
            nc.vector.tensor_copy(ones_row[:], ones_row_f[:])
            iota_sb = cpool.tile([128, S // 128], F32, name="iota", tag="iota")
            nc.scalar.dma_start(iota_sb[:], iota_kt[:])
            bq_sb = cpool.tile([128, D // 128], F32, name="bq", tag="bq")
            nc.scalar.dma_start(bq_sb[:], bq[:])
            bk_sb = cpool.tile([128, D // 128], F32, name="bk", tag="bk")
            nc.scalar.dma_start(bk_sb[:], bk[:])
            bo2_sb = cpool.tile([128, D // 128], F32, name="bo2", tag="bo2")
            nc.scalar.dma_start(bo2_sb[:], bo2[:])
            bf_sb = cpool.tile([128, M // 128], F32, name="bf", tag="bf")
            nc.scalar.dma_start(bf_sb[:], bfT[:])
            qpos_row = cpool.tile([1, 4 * CH], F32R, name="qpos_row", tag="qpos_row")
            nc.scalar.dma_start(qpos_row[:], qpos[:])

            # qposB is broadcast later (just before P2): putting its matmul
            # here would head-block the in-order PE stream on the qpos DMA
            qposB = cpool.tile([128, 4 * CH], F32, name="qposB", tag="qposB")

            # ------------- long-lived tiles: one pool spanning P1..P4 -------
            with (
                tc.tile_pool(name="main", bufs=1) as mp,
                tc.tile_pool(name="dram", bufs=1, space="DRAM") as dram,
            ):
                kT = [mp.tile([128, S], F16, name=f"kT{i}", tag=f"kT{i}") for i in range(NDT)]
                vt = [mp.tile([128, D], F16, name=f"v{k}", tag=f"v{k}") for k in range(16)]
                wq_sb = [mp.tile([128, D], F16, name=f"wq{i}", tag=f"wq{i}") for i in range(NDT)]
                wo_sb = [mp.tile([128, D], F16, name=f"wo{i}", tag=f"wo{i}") for i in range(NDT)]
                xq = [mp.tile([128, 4 * CH], F16, name=f"xq{i}", tag=f"xq{i}") for i in range(NDT)]
                qT = [
                    [mp.tile([128, 512], F16, name=f"qT{dt_}_{qb}", tag=f"qT{dt_}_{qb}") for qb in range(2)]
                    for dt_ in range(NDT)
                ]
                attnT = [
                    [mp.tile([128, 512], F16, name=f"at{dt_}_{qb}", tag=f"at{dt_}_{qb}") for qb in range(2)]
                    for dt_ in range(NDT)
                ]
                # DRAM bounce buffers for the pairwise K AllGather
                kv_in = dram.tile([D, S // 2], F16, name="kv_in", tag="kv_in")
                kv_out = dram.tile([2, D, S // 2], F16, name="kv_out", tag="kv_out")

                # ---------------- P1: K/V projections (fused x pass) --------
                with tc.tile_pool(name="p1a", bufs=1) as p1a:
                    wk_sb = [p1a.tile([128, D], F16, name=f"wk{i}", tag=f"wk{i}") for i in range(NDT)]
                    wv_sb = [p1a.tile([128, D], F16, name=f"wv{i}", tag=f"wv{i}") for i in range(NDT)]
                    # weight streams on gpsimd, independent of x on sync.
                    # Order = first-use order: wk h0 (og=0 groups), wk h1,
                    # wv, then wq / wo / xq (needed much later).
                    for h in range(2):
                        for i in range(NDT):
                            nc.gpsimd.dma_start(
                                wk_sb[i][:, ts(h, 512)], wkT[ts(i, 128), ts(h, 512)]
                            )
                    # wq right after wk: Q projection runs second (it fills
                    # the window while wv/xa still stream in)
                    for h in range(2):
                        for i in range(NDT):
                            nc.gpsimd.dma_start(
                                wq_sb[i][:, ts(h, 512)], wqT[ts(i, 128), ts(h, 512)]
                            )
                    xbufs = {}

                    def load_x(tb, src, xtag):
                        xblk = [p1a.tile([128, 512], F16, name=f"{xtag}{i}", tag=f"{xtag}{i}", bufs=2) for i in range(NDT)]
                        for i in range(NDT):
                            nc.sync.dma_start(xblk[i][:], src[ts(i, 128), ts(tb, 512)])
                        xbufs[(xtag, tb)] = xblk

                    def k_own_pass(tb):
                        # K projection over this core's OWN half of the
                        # sequence only; the peer half arrives via AllGather.
                        # Results stage into kT[:, 0:1024] (overwritten with
                        # the correctly-ordered gather output later).
                        # i-outer across 4 parallel PSUM banks, so the first
                        # matmul only needs wk[0] h0 + xblk[0]
                        xblk = xbufs.pop(("xa", tb))
                        for og in range(2):
                            ps4 = [
                                pspool.tile([128, 512], F32, name=f"at{j}", tag=f"at{j}", bufs=1)
                                for j in range(4)
                            ]
                            for i in range(NDT):
                                for j in range(4):
                                    nc.tensor.matmul(
                                        ps4[j][:],
                                        wk_sb[i][:, ts(og * 4 + j, 128)],
                                        xblk[i][:],
                                        start=(i == 0), stop=(i == NDT - 1),
                                    )
                            for j in range(4):
                                ot = og * 4 + j
                                nc.scalar.activation(
                                    kT[ot][:, ts(tb, 512)], ps4[j][:], AF.Identity,
                                    bias=bk_sb[:, ot : ot + 1],
                                )

                    def v_pass(tb):
                        # token-major; ob-outer so the first half only needs
                        # the h0 half of wv (which lands first)
                        xblk = xbufs.pop(("xa", tb))
                        for ob in range(2):
                            for tt in range(4):
                                ps = pspool.tile([128, 512], F32, name="mm", tag="mm", bufs=3)
                                for i in range(NDT):
                                    nc.tensor.matmul(
                                        ps[:],
                                        xblk[i][:, ts(tt, 128)],
                                        wv_sb[i][:, ts(ob, 512)],
                                        start=(i == 0), stop=(i == NDT - 1),
                                    )
                                nc.scalar.activation(
                                    vt[tb * 4 + tt][:, ts(ob, 512)], ps[:], AF.Identity
                                )

                    # K (own half) first; its gather runs while Q/V compute
                    load_x(0, xkT, "xa")
                    load_x(1, xkT, "xa")
                    # xq loads follow the xk stream on sync (Q runs second)
                    for i in range(NDT):
                        for h in range(2):
                            nc.sync.dma_start(
                                xq[i][:, ts(h, 512)], xoT[ts(i, 128), ts(h, 512)]
                            )
                    k_own_pass(0)
                    k_own_pass(1)
                    # stage own K^T half to DRAM and gather the pair
                    for ot in range(NDT):
                        nc.scalar.dma_start(kv_in[ts(ot, 128), :], kT[ot][:, 0 : S // 2])
                    nc.gpsimd.collective_compute(
                        "AllGather",
                        mybir.AluOpType.bypass,
                        replica_groups=[[0, 1], [2, 3], [4, 5], [6, 7]],
                        ins=[kv_in[:].opt()],
                        outs=[kv_out[:].opt()],
                    )
                    # wv / wo weight streams follow the trigger on gpsimd
                    for h in range(2):
                        for i in range(NDT):
                            nc.gpsimd.dma_start(
                                wv_sb[i][:, ts(h, 512)], wvT[ts(i, 128), ts(h, 512)]
                            )
                    for h in range(2):
                        for i in range(NDT):
                            nc.gpsimd.dma_start(
                                wo_sb[i][:, ts(h, 512)], woT[ts(i, 128), ts(h, 512)]
                            )

                    # ---------------- Q projection (fills the wv wait) ------
                    for qb in range(2):
                        for ot in range(NDT):
                            ps = pspool.tile([128, 512], F32, name="mm", tag="mm", bufs=3)
                            for i in range(NDT):
                                nc.tensor.matmul(
                                    ps[:], wq_sb[i][:, ts(ot, 128)], xq[i][:, ts(qb, 512)],
                                    start=(i == 0), stop=(i == NDT - 1),
                                )
                            nc.scalar.activation(
                                qT[ot][qb][:], ps[:], AF.Identity, bias=bq_sb[:, ot : ot + 1]
                            )

                    # V over the full sequence (still data-parallel-redundant)
                    load_x(0, xT, "xa")
                    load_x(1, xT, "xa")
                    v_pass(0)
                    load_x(2, xT, "xa")
                    v_pass(1)
                    load_x(3, xT, "xa")
                    v_pass(2)
                    v_pass(3)

                # gathered K^T readback in true token order (rank0 = tokens
                # 0:1024 first: attention slots 0/1 only need those)
                for r in range(2):
                    for i in range(NDT):
                        nc.sync.dma_start(
                            kT[i][:, r * (S // 2) : (r + 1) * (S // 2)],
                            kv_out[r, ts(i, 128), :],
                        )

                # broadcast qpos to 128 partitions via ones outer-product
                for i in range(4 * CH // 512):
                    bc_ps = pspool.tile([128, 512], F32, name="small", tag="small", bufs=1)
                    nc.tensor.matmul(
                        bc_ps[:], ones_row[:], qpos_row[:, ts(i, 512)],
                        start=True, stop=True,
                    )
                    nc.scalar.activation(qposB[:, ts(i, 512)], bc_ps[:], AF.Identity)

                # ---------------- P2: attention (4 slots of 256 queries) ----
                with tc.tile_pool(name="p2", bufs=1) as p2:
                    for sl in range(NSLOT):
                        nkt = NKT[sl]
                        qmov = qT_slice = None  # doc: moving = qT[i][sl//2][:, (sl%2)*256:+256]
                        pt = [
                            p2.tile([128, CH], F16, name=f"pt{k}", tag=f"pt{k}", bufs=1)
                            for k in range(nkt)
                        ]
                        dn_ps = pspool.tile([1, CH], F32, name="small", tag="small", bufs=1)
                        for k in range(nkt):
                            ps = pspool.tile([128, CH], F32, name="mm", tag="mm", bufs=3)
                            for i in range(NDT):
                                nc.tensor.matmul(
                                    ps[:],
                                    kT[i][:, ts(k, 128)],
                                    qT[i][sl // 2][:, ts(sl % 2, CH)],
                                    start=(i == 0), stop=(i == NDT - 1),
                                )
                            masked = k >= nkt - 4
                            if masked:
                                praw = p2.tile([128, CH], F16, name="praw", tag="praw", bufs=2)
                                nc.scalar.activation(
                                    praw[:], ps[:], AF.Exp, scale=1.0 / 32.0
                                )
                                msk = p2.tile([128, CH], F16, name="msk", tag="msk", bufs=2)
                                nc.vector.tensor_scalar(
                                    out=msk[:],
                                    in0=qposB[:, ts(sl, CH)],
                                    scalar1=iota_sb[:, k : k + 1],
                                    scalar2=None,
                                    op0=ALU.is_ge,
                                )
                                nc.vector.tensor_tensor(
                                    out=pt[k][:], in0=praw[:], in1=msk[:],
                                    op=ALU.mult,
                                )
                            else:
                                nc.scalar.activation(
                                    pt[k][:], ps[:], AF.Exp, scale=1.0 / 32.0
                                )
                            nc.tensor.matmul(
                                dn_ps[:], ones_col_bf[:], pt[k][:],
                                start=(k == 0), stop=(k == nkt - 1),
                            )
                        # broadcast denom to 128 partitions, then reciprocal
                        # full-width (a [1,256] DVE op is ~10x slower than the
                        # [128,256] one)
                        dn_r = p2.tile([1, CH], F32R, name="dn_r", tag="dn_r", bufs=2)
                        nc.vector.tensor_copy(dn_r[:], dn_ps[:])
                        rb_ps = pspool.tile([128, CH], F32, name="small", tag="small", bufs=1)
                        nc.tensor.matmul(
                            rb_ps[:], ones_row[:], dn_r[:], start=True, stop=True
                        )
                        recipB = p2.tile([128, CH], F32, name="recipB", tag="recipB", bufs=1)
                        nc.vector.reciprocal(recipB[:], rb_ps[:])
                        # attn^T = (P @ V)^T scaled by 1/denom, two 4-bank passes
                        for half in range(2):
                            for d4 in range(4):
                                d_ = half * 4 + d4
                                aps = pspool.tile([128, CH], F32, name=f"at{d4}", tag=f"at{d4}", bufs=1)
                                for k in range(nkt):
                                    nc.tensor.matmul(
                                        aps[:],
                                        vt[k][:, ts(d_, 128)],
                                        pt[k][:],
                                        start=(k == 0), stop=(k == nkt - 1),
                                    )
                                nc.vector.tensor_tensor(
                                    out=attnT[d_][sl // 2][:, ts(sl % 2, CH)],
                                    in0=aps[:], in1=recipB[:],
                                    op=ALU.mult,
                                )

                # ---------------- P3 + P4 ----------------
                with tc.tile_pool(name="p34", bufs=1) as p34:
                    outT = [
                        [p34.tile([128, 512], F16, name=f"oT{dt_}_{qb}", tag=f"oT{dt_}_{qb}") for qb in range(2)]
                        for dt_ in range(NDT)
                    ]
                    # prefetch the first two FFN weight blocks during P3
                    wfb_pool = p34
                    for qb in range(2):
                        for ot in range(NDT):
                            ps = pspool.tile([128, 512], F32, name="mm", tag="mm", bufs=3)
                            for i in range(NDT):
                                nc.tensor.matmul(
                                    ps[:],
                                    wo_sb[i][:, ts(ot, 128)],
                                    attnT[i][qb][:],
                                    start=(i == 0), stop=(i == NDT - 1),
                                )
                            nc.scalar.activation(
                                outT[ot][qb][:], ps[:], AF.Identity, bias=bo2_sb[:, ot : ot + 1]
                            )

                    # ---------------- P4: FFN + GELU ----------------
                    for mb in range(M // 512):
                        wfb = [
                            wfb_pool.tile([128, 512], F16, name=f"wf{i}", tag=f"wf{i}", bufs=2)
                            for i in range(NDT)
                        ]
                        for i in range(NDT):
                            nc.gpsimd.dma_start(wfb[i][:], wfT[ts(i, 128), ts(mb, 512)])
                        for mt in range(4):
                            m = mb * 4 + mt
                            for qb in range(2):
                                ps = pspool.tile([128, 512], F32, name="mm", tag="mm", bufs=3)
                                for i in range(NDT):
                                    nc.tensor.matmul(
                                        ps[:],
                                        wfb[i][:, ts(mt, 128)],
                                        outT[i][qb][:],
                                        start=(i == 0), stop=(i == NDT - 1),
                                    )
                                st = p34.tile([128, 512], F16, name="ffstage", tag="ffstage", bufs=4)
                                nc.scalar.activation(
                                    st[:], ps[:], AF.Gelu, bias=bf_sb[:, m : m + 1]
                                )
                                # store issued by scalar right after its GELU
                                nc.scalar.dma_start(ffT[ts(m, 128), ts(qb, 512)], st[:])

    nc.compile()
    return nc


def _get_program():
    global _PROGRAM
    if _PROGRAM is None:
        _PROGRAM = _build_program()
    return _PROGRAM


def _owned_chunks(core):
    """The four 256-token chunk indices this core owns, in slot order."""
    if core % 2 == 0:
        return (0, 3, 4, 7)
    return (1, 2, 5, 6)


def _make_in_maps(x, Wq, bq, Wk, bk, Wv, bv, Wo, bo, Wf, bf):
    f32, f16 = np.float32, np.float16
    wqT = np.ascontiguousarray(Wq.T, dtype=f16)
    wkT = np.ascontiguousarray(Wk.T, dtype=f16)
    wvT = np.ascontiguousarray(Wv.T, dtype=f16)
    woT = np.ascontiguousarray(Wo.T, dtype=f16)
    wfT = np.ascontiguousarray(Wf.T, dtype=f16)
    bo2 = (Wo.astype(np.float64) @ bv.astype(np.float64) + bo.astype(np.float64))
    bo2 = np.ascontiguousarray(bo2.astype(f32).reshape(D // 128, 128).T)
    bfT = np.ascontiguousarray(bf.reshape(M // 128, 128).T, dtype=f32)
    iota = (
        np.arange(128, dtype=f32)[:, None]
        + 128.0 * np.arange(S // 128, dtype=f32)[None, :]
    )
    shared = {
        "wqT": wqT, "wkT": wkT, "wvT": wvT, "woT": woT, "wfT": wfT,
        "bq": np.ascontiguousarray(bq.reshape(D // 128, 128).T, dtype=f32),
        "bk": np.ascontiguousarray(bk.reshape(D // 128, 128).T, dtype=f32),
        "bo2": bo2,
        "bfT": bfT,
        "iota_kt": np.ascontiguousarray(iota),
    }
    in_maps = []
    for core in range(N_CORES):
        b = core // 2
        chunks = _owned_chunks(core)
        xTb = np.ascontiguousarray(x[b].T, dtype=f16)  # [D, S]
        half = core % 2  # rank within the pair: rank0 owns tokens 0:S/2
        xkT = np.ascontiguousarray(xTb[:, half * (S // 2) : (half + 1) * (S // 2)])
        xoT = np.ascontiguousarray(
            np.concatenate([xTb[:, c * CH : (c + 1) * CH] for c in chunks], axis=1)
        )
        qp = np.concatenate(
            [np.arange(c * CH, (c + 1) * CH) for c in chunks]
        ).astype(f32)[None, :]
        in_maps.append(
            {**shared, "xT": xTb, "xkT": xkT, "xoT": xoT,
             "qpos": np.ascontiguousarray(qp)}
        )
    return in_maps


def _run(inputs, trace=False, trace_cores=None, tmpdir=None):
    import sys

    if "/opt/trn_rl_repo" not in sys.path:
        sys.path.insert(0, "/opt/trn_rl_repo")
    from concourse.bass_utils import run_bass_kernel_spmd

    nc = _get_program()
    in_maps = _make_in_maps(**inputs)
    res = run_bass_kernel_spmd(
        nc, in_maps, list(range(N_CORES)), trace=trace, trace_cores=trace_cores,
        tmpdir=tmpdir,
    )
    out = np.empty((B, S, M), dtype=np.float32)
    for core in range(N_CORES):
        b = core // 2
        chunks = _owned_chunks(core)
        ffT = res.results[core]["ffT"]  # [M, 1024] fp16
        for sl, c in enumerate(chunks):
            out[b, c * CH : (c + 1) * CH] = (
                ffT[:, sl * CH : (sl + 1) * CH].T.astype(np.float32)
            )
    return out, res


def kernel(**inputs):
    out, _ = _run(inputs)
    return out


# revision 29
# speedup vs baseline: 1.0122x; 1.0122x over previous
"""Decoder block (single-head causal attention + GELU FFN) on 8 TRN2 NeuronCores.

Sharding: pure data parallel, no collectives. Core c handles batch b = c//2 and
1024 query tokens of that batch, chosen as four 256-token chunks that balance
the causal-attention workload:
  even cores (half 0): chunks 0, 3, 4, 7
  odd  cores (half 1): chunks 1, 2, 5, 6
The slot pairing makes the static k-tile counts per slot (4, 8, 12, 16) cover
both cores' needs with minimal waste (ideal is (2..16); the gap is zeroed by
the data-driven qpos mask). The SPMD program is identical on every core; all
per-core differences are data (which tokens are in xq, qpos values that drive
on-chip causal-mask creation).

Performance structure:
  - every matmul operand is fp16 (same PE rate as fp32r, half the DMA/SBUF)
  - K and V projections fused into one pass over x (x read once)
  - K^T and V stay resident in SBUF (no DRAM round-trips)
  - DMA issue split across engines: weights on gpsimd, x on sync, stores on
    scalar, consts on scalar — so tile back-pressure in one stream never
    delays another
  - all long-lived tiles (incl. wq/wo/xq) live in one pool so their loads are
    not gated on earlier phases' SBUF frees
  - scalar engine runs only Identity in P1 and only Exp in P2 (activation
    table reloads cost 1.3us each)
"""

import numpy as np

D = 1024  # model dim
S = 2048  # sequence length
B = 4  # batch
M = 4096  # FFN dim
CH = 256  # q chunk (slot) size
NSLOT = 4  # q slots per core
NDT = D // 128  # 8 d-tiles
N_CORES = 8
NKT = [4, 8, 12, 16]  # k-tiles per slot (static max over the two paired cores)

_PROGRAM = None  # cached compiled program


def _build_program():
    import sys

    if "/opt/trn_rl_repo" not in sys.path:
        sys.path.insert(0, "/opt/trn_rl_repo")
    import concourse.bass as bass
    import concourse.tile as tile
    import concourse.mybir as mybir
    from concourse import bacc
    from concourse.bass import ts

    dt = mybir.dt
    AF = mybir.ActivationFunctionType
    ALU = mybir.AluOpType
    F32, F32R, F16 = dt.float32, dt.float32r, dt.float16

    nc = bacc.Bacc("TRN2", target_bir_lowering=False, debug=False, num_devices=8)

    # ---------------- DRAM I/O ----------------
    xT = nc.dram_tensor("xT", [D, S], F16, kind="ExternalInput").ap()
    xkT = nc.dram_tensor("xkT", [D, S // 2], F16, kind="ExternalInput").ap()
    xoT = nc.dram_tensor("xoT", [D, 4 * CH], F16, kind="ExternalInput").ap()
    wqT = nc.dram_tensor("wqT", [D, D], F16, kind="ExternalInput").ap()
    wkT = nc.dram_tensor("wkT", [D, D], F16, kind="ExternalInput").ap()
    wvT = nc.dram_tensor("wvT", [D, D], F16, kind="ExternalInput").ap()
    woT = nc.dram_tensor("woT", [D, D], F16, kind="ExternalInput").ap()
    wfT = nc.dram_tensor("wfT", [D, M], F16, kind="ExternalInput").ap()
    bq = nc.dram_tensor("bq", [128, D // 128], F32, kind="ExternalInput").ap()
    bk = nc.dram_tensor("bk", [128, D // 128], F32, kind="ExternalInput").ap()
    bo2 = nc.dram_tensor("bo2", [128, D // 128], F32, kind="ExternalInput").ap()
    bfT = nc.dram_tensor("bfT", [128, M // 128], F32, kind="ExternalInput").ap()
    qpos = nc.dram_tensor("qpos", [1, 4 * CH], F32R, kind="ExternalInput").ap()
    iota_kt = nc.dram_tensor("iota_kt", [128, S // 128], F32, kind="ExternalInput").ap()
    ffT = nc.dram_tensor("ffT", [M, 4 * CH], F16, kind="ExternalOutput").ap()

    with tile.TileContext(nc) as tc:
        with (
            tc.tile_pool(name="const", bufs=1) as cpool,
            tc.tile_pool(name="psum", bufs=1, space="PSUM") as pspool,
        ):
            # ---------------- constants (scalar engine issues these) --------
            ones_col_bf = cpool.tile([128, 1], F16, name="ones_col_bf", tag="ones_col_bf")
            nc.vector.memset(ones_col_bf[:], 1.0)
            ones_row_f = cpool.tile([1, 128], F32, name="ones_row_f", tag="ones_row_f")
            nc.vector.memset(ones_row_f[:], 1.0)
            ones_row = cpool.tile([1, 128], F32R, name="ones_row", tag="ones_row")
            nc.vector.tensor_copy(ones_row[:], ones_row_f[:])
            iota_sb = cpool.tile([128, S // 128], F32, name="iota", tag="iota")
            nc.scalar.dma_start(iota_sb[:], iota_kt[:])
            bq_sb = cpool.tile([128, D // 128], F32, name="bq", tag="bq")
            nc.scalar.dma_start(bq_sb[:], bq[:])
            bk_sb = cpool.tile([128, D // 128], F32, name="bk", tag="bk")
            nc.scalar.dma_start(bk_sb[:], bk[:])
            bo2_sb = cpool.tile([128, D // 128], F32, name="bo2", tag="bo2")
            nc.scalar.dma_start(bo2_sb[:], bo2[:])
            bf_sb = cpool.tile([128, M // 128], F32, name="bf", tag="bf")
            nc.scalar.dma_start(bf_sb[:], bfT[:])
            qpos_row = cpool.tile([1, 4 * CH], F32R, name="qpos_row", tag="qpos_row")
            nc.scalar.dma_start(qpos_row[:], qpos[:])

            # qposB is broadcast later (just before P2): putting its matmul
            # here would head-block the in-order PE stream on the qpos DMA
            qposB = cpool.tile([128, 4 * CH], F32, name="qposB", tag="qposB")

            # ------------- long-lived tiles: one pool spanning P1..P4 -------
            with (
                tc.tile_pool(name="main", bufs=1) as mp,
                tc.tile_pool(name="dram", bufs=1, space="DRAM") as dram,
            ):
                kT = [mp.tile([128, S], F16, name=f"kT{i}", tag=f"kT{i}") for i in range(NDT)]
                vt = [mp.tile([128, D], F16, name=f"v{k}", tag=f"v{k}") for k in range(16)]
                wq_sb = [mp.tile([128, D], F16, name=f"wq{i}", tag=f"wq{i}") for i in range(NDT)]
                wo_sb = [mp.tile([128, D], F16, name=f"wo{i}", tag=f"wo{i}") for i in range(NDT)]
                xq = [mp.tile([128, 4 * CH], F16, name=f"xq{i}", tag=f"xq{i}") for i in range(NDT)]
                qT = [
                    [mp.tile([128, 512], F16, name=f"qT{dt_}_{qb}", tag=f"qT{dt_}_{qb}") for qb in range(2)]
                    for dt_ in range(NDT)
                ]
                attnT = [
                    [mp.tile([128, 512], F16, name=f"at{dt_}_{qb}", tag=f"at{dt_}_{qb}") for qb in range(2)]
                    for dt_ in range(NDT)
                ]
                # DRAM bounce buffers for the pairwise K AllGather
                kv_in = dram.tile([D, S // 2], F16, name="kv_in", tag="kv_in")
                kv_out = dram.tile([2, D, S // 2], F16, name="kv_out", tag="kv_out")

                # ---------------- P1: K/V projections (fused x pass) --------
                with tc.tile_pool(name="p1a", bufs=1) as p1a:
                    wk_sb = [p1a.tile([128, D], F16, name=f"wk{i}", tag=f"wk{i}") for i in range(NDT)]
                    wv_sb = [p1a.tile([128, D], F16, name=f"wv{i}", tag=f"wv{i}") for i in range(NDT)]
                    # weight streams on gpsimd, independent of x on sync.
                    # Order = first-use order: wk h0 (og=0 groups), wk h1,
                    # wv, then wq / wo / xq (needed much later).
                    for h in range(2):
                        for i in range(NDT):
                            nc.gpsimd.dma_start(
                                wk_sb[i][:, ts(h, 512)], wkT[ts(i, 128), ts(h, 512)]
                            )
                    # wq right after wk: Q projection runs second (it fills
                    # the window while wv/xa still stream in)
                    for h in range(2):
                        for i in range(NDT):
                            nc.gpsimd.dma_start(
                                wq_sb[i][:, ts(h, 512)], wqT[ts(i, 128), ts(h, 512)]
                            )
                    xbufs = {}

                    def load_x(tb, src, xtag):
                        xblk = [p1a.tile([128, 512], F16, name=f"{xtag}{i}", tag=f"{xtag}{i}", bufs=2) for i in range(NDT)]
                        for i in range(NDT):
                            nc.sync.dma_start(xblk[i][:], src[ts(i, 128), ts(tb, 512)])
                        xbufs[(xtag, tb)] = xblk

                    def k_own_pass(tb):
                        # K projection over this core's OWN half of the
                        # sequence only; the peer half arrives via AllGather.
                        # Results stage into kT[:, 0:1024] (overwritten with
                        # the correctly-ordered gather output later).
                        # i-outer across 4 parallel PSUM banks, so the first
                        # matmul only needs wk[0] h0 + xblk[0]
                        xblk = xbufs.pop(("xa", tb))
                        for og in range(2):
                            ps4 = [
                                pspool.tile([128, 512], F32, name=f"at{j}", tag=f"at{j}", bufs=1)
                                for j in range(4)
                            ]
                            for i in range(NDT):
                                for j in range(4):
                                    nc.tensor.matmul(
                                        ps4[j][:],
                                        wk_sb[i][:, ts(og * 4 + j, 128)],
                                        xblk[i][:],
                                        start=(i == 0), stop=(i == NDT - 1),
                                    )
                            for j in range(4):
                                ot = og * 4 + j
                                nc.scalar.activation(
                                    kT[ot][:, ts(tb, 512)], ps4[j][:], AF.Identity,
                                    bias=bk_sb[:, ot : ot + 1],
                                )

                    def v_pass(tb):
                        # token-major, i-outer across 2 banks per tt
                        xblk = xbufs.pop(("xa", tb))
                        for tt in range(4):
                            ps2 = [
                                pspool.tile([128, 512], F32, name=f"mm{ob}", tag="mm", bufs=3)
                                for ob in range(2)
                            ]
                            for i in range(NDT):
                                for ob in range(2):
                                    nc.tensor.matmul(
                                        ps2[ob][:],
                                        xblk[i][:, ts(tt, 128)],
                                        wv_sb[i][:, ts(ob, 512)],
                                        start=(i == 0), stop=(i == NDT - 1),
                                    )
                            for ob in range(2):
                                nc.scalar.activation(
                                    vt[tb * 4 + tt][:, ts(ob, 512)], ps2[ob][:], AF.Identity
                                )

                    # K (own half) first; its gather runs while Q/V compute
                    load_x(0, xkT, "xa")
                    load_x(1, xkT, "xa")
                    # xq loads follow the xk stream on sync (Q runs second)
                    for i in range(NDT):
                        for h in range(2):
                            nc.sync.dma_start(
                                xq[i][:, ts(h, 512)], xoT[ts(i, 128), ts(h, 512)]
                            )
                    k_own_pass(0)
                    k_own_pass(1)
                    # stage own K^T half to DRAM and gather the pair
                    for ot in range(NDT):
                        nc.scalar.dma_start(kv_in[ts(ot, 128), :], kT[ot][:, 0 : S // 2])
                    nc.gpsimd.collective_compute(
                        "AllGather",
                        mybir.AluOpType.bypass,
                        replica_groups=[[0, 1], [2, 3], [4, 5], [6, 7]],
                        ins=[kv_in[:].opt()],
                        outs=[kv_out[:].opt()],
                    )
                    # wv / wo weight streams follow the trigger on gpsimd
                    for h in range(2):
                        for i in range(NDT):
                            nc.gpsimd.dma_start(
                                wv_sb[i][:, ts(h, 512)], wvT[ts(i, 128), ts(h, 512)]
                            )
                    for h in range(2):
                        for i in range(NDT):
                            nc.gpsimd.dma_start(
                                wo_sb[i][:, ts(h, 512)], woT[ts(i, 128), ts(h, 512)]
                            )

                    # ---------------- Q projection (fills the wv wait) ------
                    for qb in range(2):
                        for ot in range(NDT):
                            ps = pspool.tile([128, 512], F32, name="mm", tag="mm", bufs=3)
                            for i in range(NDT):
                                nc.tensor.matmul(
                                    ps[:], wq_sb[i][:, ts(ot, 128)], xq[i][:, ts(qb, 512)],
                                    start=(i == 0), stop=(i == NDT - 1),
                                )
                            nc.scalar.activation(
                                qT[ot][qb][:], ps[:], AF.Identity, bias=bq_sb[:, ot : ot + 1]
                            )

                    # V over the full sequence (still data-parallel-redundant)
                    load_x(0, xT, "xa")
                    load_x(1, xT, "xa")
                    v_pass(0)
                    load_x(2, xT, "xa")
                    v_pass(1)
                    load_x(3, xT, "xa")
                    v_pass(2)
                    v_pass(3)

                # gathered K^T readback in true token order (rank0 = tokens
                # 0:1024 first: attention slots 0/1 only need those)
                for r in range(2):
                    for i in range(NDT):
                        nc.sync.dma_start(
                            kT[i][:, r * (S // 2) : (r + 1) * (S // 2)],
                            kv_out[r, ts(i, 128), :],
                        )

                # broadcast qpos to 128 partitions via ones outer-product
                for i in range(4 * CH // 512):
                    bc_ps = pspool.tile([128, 512], F32, name="small", tag="small", bufs=1)
                    nc.tensor.matmul(
                        bc_ps[:], ones_row[:], qpos_row[:, ts(i, 512)],
                        start=True, stop=True,
                    )
                    nc.scalar.activation(qposB[:, ts(i, 512)], bc_ps[:], AF.Identity)

                # ---------------- P2: attention (4 slots of 256 queries) ----
                with tc.tile_pool(name="p2", bufs=1) as p2:
                    for sl in range(NSLOT):
                        nkt = NKT[sl]
                        qmov = qT_slice = None  # doc: moving = qT[i][sl//2][:, (sl%2)*256:+256]
                        pt = [
                            p2.tile([128, CH], F16, name=f"pt{k}", tag=f"pt{k}", bufs=1)
                            for k in range(nkt)
                        ]
                        dn_ps = pspool.tile([1, CH], F32, name="small", tag="small", bufs=1)
                        for k in range(nkt):
                            ps = pspool.tile([128, CH], F32, name="mm", tag="mm", bufs=3)
                            for i in range(NDT):
                                nc.tensor.matmul(
                                    ps[:],
                                    kT[i][:, ts(k, 128)],
                                    qT[i][sl // 2][:, ts(sl % 2, CH)],
                                    start=(i == 0), stop=(i == NDT - 1),
                                )
                            masked = k >= nkt - 4
                            if masked:
                                praw = p2.tile([128, CH], F16, name="praw", tag="praw", bufs=2)
                                nc.scalar.activation(
                                    praw[:], ps[:], AF.Exp, scale=1.0 / 32.0
                                )
                                msk = p2.tile([128, CH], F16, name="msk", tag="msk", bufs=2)
                                nc.vector.tensor_scalar(
                                    out=msk[:],
                                    in0=qposB[:, ts(sl, CH)],
                                    scalar1=iota_sb[:, k : k + 1],
                                    scalar2=None,
                                    op0=ALU.is_ge,
                                )
                                nc.vector.tensor_tensor(
                                    out=pt[k][:], in0=praw[:], in1=msk[:],
                                    op=ALU.mult,
                                )
                            else:
                                nc.scalar.activation(
                                    pt[k][:], ps[:], AF.Exp, scale=1.0 / 32.0
                                )
                            nc.tensor.matmul(
                                dn_ps[:], ones_col_bf[:], pt[k][:],
                                start=(k == 0), stop=(k == nkt - 1),
                            )
                        # broadcast denom to 128 partitions, then reciprocal
                        # full-width (a [1,256] DVE op is ~10x slower than the
                        # [128,256] one)
                        dn_r = p2.tile([1, CH], F32R, name="dn_r", tag="dn_r", bufs=2)
                        nc.vector.tensor_copy(dn_r[:], dn_ps[:])
                        rb_ps = pspool.tile([128, CH], F32, name="small", tag="small", bufs=1)
                        nc.tensor.matmul(
                            rb_ps[:], ones_row[:], dn_r[:], start=True, stop=True
                        )
                        recipB = p2.tile([128, CH], F32, name="recipB", tag="recipB", bufs=1)
                        nc.vector.reciprocal(recipB[:], rb_ps[:])
                        # attn^T = (P @ V)^T scaled by 1/denom, two 4-bank passes
                        for half in range(2):
                            for d4 in range(4):
                                d_ = half * 4 + d4
                                aps = pspool.tile([128, CH], F32, name=f"at{d4}", tag=f"at{d4}", bufs=1)
                                for k in range(nkt):
                                    nc.tensor.matmul(
                                        aps[:],
                                        vt[k][:, ts(d_, 128)],
                                        pt[k][:],
                                        start=(k == 0), stop=(k == nkt - 1),
                                    )
                                nc.vector.tensor_tensor(
                                    out=attnT[d_][sl // 2][:, ts(sl % 2, CH)],
                                    in0=aps[:], in1=recipB[:],
                                    op=ALU.mult,
                                )

                # ---------------- P3 + P4 ----------------
                with tc.tile_pool(name="p34", bufs=1) as p34:
                    outT = [
                        [p34.tile([128, 512], F16, name=f"oT{dt_}_{qb}", tag=f"oT{dt_}_{qb}") for qb in range(2)]
                        for dt_ in range(NDT)
                    ]
                    # prefetch the first two FFN weight blocks during P3
                    wfb_pool = p34
                    for qb in range(2):
                        for ot in range(NDT):
                            ps = pspool.tile([128, 512], F32, name="mm", tag="mm", bufs=3)
                            for i in range(NDT):
                                nc.tensor.matmul(
                                    ps[:],
                                    wo_sb[i][:, ts(ot, 128)],
                                    attnT[i][qb][:],
                                    start=(i == 0), stop=(i == NDT - 1),
                                )
                            nc.scalar.activation(
                                outT[ot][qb][:], ps[:], AF.Identity, bias=bo2_sb[:, ot : ot + 1]
                            )

                    # ---------------- P4: FFN + GELU ----------------
                    for mb in range(M // 512):
                        wfb = [
                            wfb_pool.tile([128, 512], F16, name=f"wf{i}", tag=f"wf{i}", bufs=2)
                            for i in range(NDT)
                        ]
                        for i in range(NDT):
                            nc.gpsimd.dma_start(wfb[i][:], wfT[ts(i, 128), ts(mb, 512)])
                        for mt in range(4):
                            m = mb * 4 + mt
                            for qb in range(2):
                                ps = pspool.tile([128, 512], F32, name="mm", tag="mm", bufs=3)
                                for i in range(NDT):
                                    nc.tensor.matmul(
                                        ps[:],
                                        wfb[i][:, ts(mt, 128)],
                                        outT[i][qb][:],
                                        start=(i == 0), stop=(i == NDT - 1),
                                    )
                                st = p34.tile([128, 512], F16, name="ffstage", tag="ffstage", bufs=4)
                                nc.scalar.activation(
                                    st[:], ps[:], AF.Gelu, bias=bf_sb[:, m : m + 1]
                                )
                                # store issued by scalar right after its GELU
                                nc.scalar.dma_start(ffT[ts(m, 128), ts(qb, 512)], st[:])

    nc.compile()
    return nc


def _get_program():
    global _PROGRAM
    if _PROGRAM is None:
        _PROGRAM = _build_program()
    return _PROGRAM


def _owned_chunks(core):
    """The four 256-token chunk indices this core owns, in slot order."""
    if core % 2 == 0:
        return (0, 3, 4, 7)
    return (1, 2, 5, 6)


def _make_in_maps(x, Wq, bq, Wk, bk, Wv, bv, Wo, bo, Wf, bf):
    f32, f16 = np.float32, np.float16
    wqT = np.ascontiguousarray(Wq.T, dtype=f16)
    wkT = np.ascontiguousarray(Wk.T, dtype=f16)
    wvT = np.ascontiguousarray(Wv.T, dtype=f16)
    woT = np.ascontiguousarray(Wo.T, dtype=f16)
    wfT = np.ascontiguousarray(Wf.T, dtype=f16)
    bo2 = (Wo.astype(np.float64) @ bv.astype(np.float64) + bo.astype(np.float64))
    bo2 = np.ascontiguousarray(bo2.astype(f32).reshape(D // 128, 128).T)
    bfT = np.ascontiguousarray(bf.reshape(M // 128, 128).T, dtype=f32)
    iota = (
        np.arange(128, dtype=f32)[:, None]
        + 128.0 * np.arange(S // 128, dtype=f32)[None, :]
    )
    shared = {
        "wqT": wqT, "wkT": wkT, "wvT": wvT, "woT": woT, "wfT": wfT,
        "bq": np.ascontiguousarray(bq.reshape(D // 128, 128).T, dtype=f32),
        "bk": np.ascontiguousarray(bk.reshape(D // 128, 128).T, dtype=f32),
        "bo2": bo2,
        "bfT": bfT,
        "iota_kt": np.ascontiguousarray(iota),
    }
    in_maps = []
    for core in range(N_CORES):
        b = core // 2
        chunks = _owned_chunks(core)
        xTb = np.ascontiguousarray(x[b].T, dtype=f16)  # [D, S]
        half = core % 2  # rank within the pair: rank0 owns tokens 0:S/2
        xkT = np.ascontiguousarray(xTb[:, half * (S // 2) : (half + 1) * (S // 2)])
        xoT = np.ascontiguousarray(
            np.concatenate([xTb[:, c * CH : (c + 1) * CH] for c in chunks], axis=1)
        )
        qp = np.concatenate(
            [np.arange(c * CH, (c + 1) * CH) for c in chunks]
        ).astype(f32)[None, :]
        in_maps.append(
            {**shared, "xT": xTb, "xkT": xkT, "xoT": xoT,
             "qpos": np.ascontiguousarray(qp)}
        )
    return in_maps


def _run(inputs, trace=False, trace_cores=None, tmpdir=None):
    import sys

    if "/opt/trn_rl_repo" not in sys.path:
        sys.path.insert(0, "/opt/trn_rl_repo")
    from concourse.bass_utils import run_bass_kernel_spmd

    nc = _get_program()
    in_maps = _make_in_maps(**inputs)
    res = run_bass_kernel_spmd(
        nc, in_maps, list(range(N_CORES)), trace=trace, trace_cores=trace_cores,
        tmpdir=tmpdir,
    )
    out = np.empty((B, S, M), dtype=np.float32)
    for core in range(N_CORES):
        b = core // 2
        chunks = _owned_chunks(core)
        ffT = res.results[core]["ffT"]  # [M, 1024] fp16
        for sl, c in enumerate(chunks):
            out[b, c * CH : (c + 1) * CH] = (
                ffT[:, sl * CH : (sl + 1) * CH].T.astype(np.float32)
            )
    return out, res


def kernel(**inputs):
    out, _ = _run(inputs)
    return out


# revision 31
# speedup vs baseline: 1.0160x; 1.0037x over previous
"""Decoder block (single-head causal attention + GELU FFN) on 8 TRN2 NeuronCores.

Sharding: pure data parallel, no collectives. Core c handles batch b = c//2 and
1024 query tokens of that batch, chosen as four 256-token chunks that balance
the causal-attention workload:
  even cores (half 0): chunks 0, 3, 4, 7
  odd  cores (half 1): chunks 1, 2, 5, 6
The slot pairing makes the static k-tile counts per slot (4, 8, 12, 16) cover
both cores' needs with minimal waste (ideal is (2..16); the gap is zeroed by
the data-driven qpos mask). The SPMD program is identical on every core; all
per-core differences are data (which tokens are in xq, qpos values that drive
on-chip causal-mask creation).

Performance structure:
  - every matmul operand is fp16 (same PE rate as fp32r, half the DMA/SBUF)
  - K and V projections fused into one pass over x (x read once)
  - K^T and V stay resident in SBUF (no DRAM round-trips)
  - DMA issue split across engines: weights on gpsimd, x on sync, stores on
    scalar, consts on scalar — so tile back-pressure in one stream never
    delays another
  - all long-lived tiles (incl. wq/wo/xq) live in one pool so their loads are
    not gated on earlier phases' SBUF frees
  - scalar engine runs only Identity in P1 and only Exp in P2 (activation
    table reloads cost 1.3us each)
"""

import numpy as np

D = 1024  # model dim
S = 2048  # sequence length
B = 4  # batch
M = 4096  # FFN dim
CH = 256  # q chunk (slot) size
NSLOT = 4  # q slots per core
NDT = D // 128  # 8 d-tiles
N_CORES = 8
NKT = [4, 8, 12, 16]  # k-tiles per slot (static max over the two paired cores)

_PROGRAM = None  # cached compiled program


def _build_program():
    import sys

    if "/opt/trn_rl_repo" not in sys.path:
        sys.path.insert(0, "/opt/trn_rl_repo")
    import concourse.bass as bass
    import concourse.tile as tile
    import concourse.mybir as mybir
    from concourse import bacc
    from concourse.bass import ts

    dt = mybir.dt
    AF = mybir.ActivationFunctionType
    ALU = mybir.AluOpType
    F32, F32R, F16 = dt.float32, dt.float32r, dt.float16

    nc = bacc.Bacc("TRN2", target_bir_lowering=False, debug=False, num_devices=8)

    # ---------------- DRAM I/O ----------------
    xT = nc.dram_tensor("xT", [D, S], F16, kind="ExternalInput").ap()
    xkT = nc.dram_tensor("xkT", [D, S // 2], F16, kind="ExternalInput").ap()
    xoT = nc.dram_tensor("xoT", [D, 4 * CH], F16, kind="ExternalInput").ap()
    wqT = nc.dram_tensor("wqT", [D, D], F16, kind="ExternalInput").ap()
    wkT = nc.dram_tensor("wkT", [D, D], F16, kind="ExternalInput").ap()
    wvT = nc.dram_tensor("wvT", [D, D], F16, kind="ExternalInput").ap()
    woT = nc.dram_tensor("woT", [D, D], F16, kind="ExternalInput").ap()
    wfT = nc.dram_tensor("wfT", [D, M], F16, kind="ExternalInput").ap()
    bq = nc.dram_tensor("bq", [128, D // 128], F32, kind="ExternalInput").ap()
    bk = nc.dram_tensor("bk", [128, D // 128], F32, kind="ExternalInput").ap()
    bo2 = nc.dram_tensor("bo2", [128, D // 128], F32, kind="ExternalInput").ap()
    bfT = nc.dram_tensor("bfT", [128, M // 128], F32, kind="ExternalInput").ap()
    qpos = nc.dram_tensor("qpos", [1, 4 * CH], F32R, kind="ExternalInput").ap()
    iota_kt = nc.dram_tensor("iota_kt", [128, S // 128], F32, kind="ExternalInput").ap()
    ffT = nc.dram_tensor("ffT", [M, 4 * CH], F16, kind="ExternalOutput").ap()

    with tile.TileContext(nc) as tc:
        with (
            tc.tile_pool(name="const", bufs=1) as cpool,
            tc.tile_pool(name="psum", bufs=1, space="PSUM") as pspool,
        ):
            # ---------------- constants (scalar engine issues these) --------
            ones_col_bf = cpool.tile([128, 1], F16, name="ones_col_bf", tag="ones_col_bf")
            nc.vector.memset(ones_col_bf[:], 1.0)
            ones_row_f = cpool.tile([1, 128], F32, name="ones_row_f", tag="ones_row_f")
            nc.vector.memset(ones_row_f[:], 1.0)
            ones_row = cpool.tile([1, 128], F32R, name="ones_row", tag="ones_row")
            nc.vector.tensor_copy(ones_row[:], ones_row_f[:])
            iota_sb = cpool.tile([128, S // 128], F32, name="iota", tag="iota")
            nc.scalar.dma_start(iota_sb[:], iota_kt[:])
            bq_sb = cpool.tile([128, D // 128], F32, name="bq", tag="bq")
            nc.scalar.dma_start(bq_sb[:], bq[:])
            bk_sb = cpool.tile([128, D // 128], F32, name="bk", tag="bk")
            nc.scalar.dma_start(bk_sb[:], bk[:])
            bo2_sb = cpool.tile([128, D // 128], F32, name="bo2", tag="bo2")
            nc.scalar.dma_start(bo2_sb[:], bo2[:])
            bf_sb = cpool.tile([128, M // 128], F32, name="bf", tag="bf")
            nc.scalar.dma_start(bf_sb[:], bfT[:])
            qpos_row = cpool.tile([1, 4 * CH], F32R, name="qpos_row", tag="qpos_row")
            nc.scalar.dma_start(qpos_row[:], qpos[:])

            # qposB is broadcast later (just before P2): putting its matmul
            # here would head-block the in-order PE stream on the qpos DMA
            qposB = cpool.tile([128, 4 * CH], F32, name="qposB", tag="qposB")

            # ------------- long-lived tiles: one pool spanning P1..P4 -------
            with (
                tc.tile_pool(name="main", bufs=1) as mp,
                tc.tile_pool(name="dram", bufs=1, space="DRAM") as dram,
            ):
                kT = [mp.tile([128, S], F16, name=f"kT{i}", tag=f"kT{i}") for i in range(NDT)]
                vt = [mp.tile([128, D], F16, name=f"v{k}", tag=f"v{k}") for k in range(16)]
                wq_sb = [mp.tile([128, D], F16, name=f"wq{i}", tag=f"wq{i}") for i in range(NDT)]
                wo_sb = [mp.tile([128, D], F16, name=f"wo{i}", tag=f"wo{i}") for i in range(NDT)]
                xq = [mp.tile([128, 4 * CH], F16, name=f"xq{i}", tag=f"xq{i}") for i in range(NDT)]
                qT = [
                    [mp.tile([128, 512], F16, name=f"qT{dt_}_{qb}", tag=f"qT{dt_}_{qb}") for qb in range(2)]
                    for dt_ in range(NDT)
                ]
                attnT = [
                    [mp.tile([128, 512], F16, name=f"at{dt_}_{qb}", tag=f"at{dt_}_{qb}") for qb in range(2)]
                    for dt_ in range(NDT)
                ]
                # DRAM bounce buffers for the pairwise K AllGather
                kv_in = dram.tile([D, S // 2], F16, name="kv_in", tag="kv_in")
                kv_out = dram.tile([2, D, S // 2], F16, name="kv_out", tag="kv_out")

                # ---------------- P1: K/V projections (fused x pass) --------
                with tc.tile_pool(name="p1a", bufs=1) as p1a:
                    wk_sb = [p1a.tile([128, D], F16, name=f"wk{i}", tag=f"wk{i}") for i in range(NDT)]
                    wv_sb = [p1a.tile([128, D], F16, name=f"wv{i}", tag=f"wv{i}") for i in range(NDT)]
                    # weight streams on gpsimd, independent of x on sync.
                    # Order = first-use order: wk h0 (og=0 groups), wk h1,
                    # wv, then wq / wo / xq (needed much later).
                    for h in range(2):
                        for i in range(NDT):
                            nc.gpsimd.dma_start(
                                wk_sb[i][:, ts(h, 512)], wkT[ts(i, 128), ts(h, 512)]
                            )
                    # wq right after wk: Q projection runs second (it fills
                    # the window while wv/xa still stream in)
                    for h in range(2):
                        for i in range(NDT):
                            nc.gpsimd.dma_start(
                                wq_sb[i][:, ts(h, 512)], wqT[ts(i, 128), ts(h, 512)]
                            )
                    xbufs = {}

                    def load_x(tb, src, xtag):
                        xblk = [p1a.tile([128, 512], F16, name=f"{xtag}{i}", tag=f"{xtag}{i}", bufs=2) for i in range(NDT)]
                        for i in range(NDT):
                            nc.sync.dma_start(xblk[i][:], src[ts(i, 128), ts(tb, 512)])
                        xbufs[(xtag, tb)] = xblk

                    def k_own_pass(tb):
                        # K projection over this core's OWN half of the
                        # sequence only; the peer half arrives via AllGather.
                        # Results stage into kT[:, 0:1024] (overwritten with
                        # the correctly-ordered gather output later).
                        # i-outer across 4 parallel PSUM banks, so the first
                        # matmul only needs wk[0] h0 + xblk[0]
                        xblk = xbufs.pop(("xa", tb))
                        for og in range(2):
                            ps4 = [
                                pspool.tile([128, 512], F32, name=f"at{j}", tag=f"at{j}", bufs=1)
                                for j in range(4)
                            ]
                            for i in range(NDT):
                                for j in range(4):
                                    nc.tensor.matmul(
                                        ps4[j][:],
                                        wk_sb[i][:, ts(og * 4 + j, 128)],
                                        xblk[i][:],
                                        start=(i == 0), stop=(i == NDT - 1),
                                    )
                            for j in range(4):
                                ot = og * 4 + j
                                nc.scalar.activation(
                                    kT[ot][:, ts(tb, 512)], ps4[j][:], AF.Identity,
                                    bias=bk_sb[:, ot : ot + 1],
                                )

                    def v_pass(tb):
                        # token-major, i-outer across 2 banks per tt
                        xblk = xbufs.pop(("xa", tb))
                        for tt in range(4):
                            ps2 = [
                                pspool.tile([128, 512], F32, name=f"mm{ob}", tag="mm", bufs=3)
                                for ob in range(2)
                            ]
                            for i in range(NDT):
                                for ob in range(2):
                                    nc.tensor.matmul(
                                        ps2[ob][:],
                                        xblk[i][:, ts(tt, 128)],
                                        wv_sb[i][:, ts(ob, 512)],
                                        start=(i == 0), stop=(i == NDT - 1),
                                    )
                            for ob in range(2):
                                nc.scalar.activation(
                                    vt[tb * 4 + tt][:, ts(ob, 512)], ps2[ob][:], AF.Identity
                                )

                    # K (own half) first; its gather runs while Q/V compute
                    load_x(0, xkT, "xa")
                    load_x(1, xkT, "xa")
                    # xq loads follow the xk stream on sync (Q runs second)
                    for i in range(NDT):
                        for h in range(2):
                            nc.sync.dma_start(
                                xq[i][:, ts(h, 512)], xoT[ts(i, 128), ts(h, 512)]
                            )
                    k_own_pass(0)
                    # qpos broadcast here: it fills part of the startup
                    # window where og1 waits on the wk h1 weight stream
                    for i in range(4 * CH // 512):
                        bc_ps = pspool.tile([128, 512], F32, name="small", tag="small", bufs=1)
                        nc.tensor.matmul(
                            bc_ps[:], ones_row[:], qpos_row[:, ts(i, 512)],
                            start=True, stop=True,
                        )
                        nc.scalar.activation(qposB[:, ts(i, 512)], bc_ps[:], AF.Identity)
                    k_own_pass(1)
                    # stage own K^T half to DRAM and gather the pair.
                    # Issued from sync (idle here): putting these on scalar
                    # delays the Q readouts behind them, which stalls the PE
                    # on PSUM-bank back-pressure (~4us measured)
                    for ot in range(NDT):
                        nc.sync.dma_start(kv_in[ts(ot, 128), :], kT[ot][:, 0 : S // 2])
                    nc.gpsimd.collective_compute(
                        "AllGather",
                        mybir.AluOpType.bypass,
                        replica_groups=[[0, 1], [2, 3], [4, 5], [6, 7]],
                        ins=[kv_in[:].opt()],
                        outs=[kv_out[:].opt()],
                    )
                    # wv / wo weight streams follow the trigger on gpsimd
                    for h in range(2):
                        for i in range(NDT):
                            nc.gpsimd.dma_start(
                                wv_sb[i][:, ts(h, 512)], wvT[ts(i, 128), ts(h, 512)]
                            )
                    for h in range(2):
                        for i in range(NDT):
                            nc.gpsimd.dma_start(
                                wo_sb[i][:, ts(h, 512)], woT[ts(i, 128), ts(h, 512)]
                            )

                    # ---------------- Q projection (fills the wv wait) ------
                    for qb in range(2):
                        for ot in range(NDT):
                            ps = pspool.tile([128, 512], F32, name="mm", tag="mm", bufs=3)
                            for i in range(NDT):
                                nc.tensor.matmul(
                                    ps[:], wq_sb[i][:, ts(ot, 128)], xq[i][:, ts(qb, 512)],
                                    start=(i == 0), stop=(i == NDT - 1),
                                )
                            nc.scalar.activation(
                                qT[ot][qb][:], ps[:], AF.Identity, bias=bq_sb[:, ot : ot + 1]
                            )

                    # V over the full sequence (still data-parallel-redundant)
                    load_x(0, xT, "xa")
                    load_x(1, xT, "xa")
                    v_pass(0)
                    load_x(2, xT, "xa")
                    v_pass(1)
                    load_x(3, xT, "xa")
                    v_pass(2)
                    v_pass(3)

                # gathered K^T readback in true token order (rank0 = tokens
                # 0:1024 first: attention slots 0/1 only need those)
                for r in range(2):
                    for i in range(NDT):
                        nc.sync.dma_start(
                            kT[i][:, r * (S // 2) : (r + 1) * (S // 2)],
                            kv_out[r, ts(i, 128), :],
                        )

                # ---------------- P2: attention (4 slots of 256 queries) ----
                with tc.tile_pool(name="p2", bufs=1) as p2:
                    for sl in range(NSLOT):
                        nkt = NKT[sl]
                        qmov = qT_slice = None  # doc: moving = qT[i][sl//2][:, (sl%2)*256:+256]
                        pt = [
                            p2.tile([128, CH], F16, name=f"pt{k}", tag=f"pt{k}", bufs=1)
                            for k in range(nkt)
                        ]
                        dn_ps = pspool.tile([1, CH], F32, name="small", tag="small", bufs=1)
                        for k in range(nkt):
                            ps = pspool.tile([128, CH], F32, name="mm", tag="mm", bufs=3)
                            for i in range(NDT):
                                nc.tensor.matmul(
                                    ps[:],
                                    kT[i][:, ts(k, 128)],
                                    qT[i][sl // 2][:, ts(sl % 2, CH)],
                                    start=(i == 0), stop=(i == NDT - 1),
                                )
                            masked = k >= nkt - 4
                            if masked:
                                praw = p2.tile([128, CH], F16, name="praw", tag="praw", bufs=2)
                                nc.scalar.activation(
                                    praw[:], ps[:], AF.Exp, scale=1.0 / 32.0
                                )
                                msk = p2.tile([128, CH], F16, name="msk", tag="msk", bufs=2)
                                nc.vector.tensor_scalar(
                                    out=msk[:],
                                    in0=qposB[:, ts(sl, CH)],
                                    scalar1=iota_sb[:, k : k + 1],
                                    scalar2=None,
                                    op0=ALU.is_ge,
                                )
                                nc.vector.tensor_tensor(
                                    out=pt[k][:], in0=praw[:], in1=msk[:],
                                    op=ALU.mult,
                                )
                            else:
                                nc.scalar.activation(
                                    pt[k][:], ps[:], AF.Exp, scale=1.0 / 32.0
                                )
                            nc.tensor.matmul(
                                dn_ps[:], ones_col_bf[:], pt[k][:],
                                start=(k == 0), stop=(k == nkt - 1),
                            )
                        # broadcast denom to 128 partitions, then reciprocal
                        # full-width (a [1,256] DVE op is ~10x slower than the
                        # [128,256] one)
                        dn_r = p2.tile([1, CH], F32R, name="dn_r", tag="dn_r", bufs=2)
                        nc.vector.tensor_copy(dn_r[:], dn_ps[:])
                        rb_ps = pspool.tile([128, CH], F32, name="small", tag="small", bufs=1)
                        nc.tensor.matmul(
                            rb_ps[:], ones_row[:], dn_r[:], start=True, stop=True
                        )
                        recipB = p2.tile([128, CH], F32, name="recipB", tag="recipB", bufs=1)
                        nc.vector.reciprocal(recipB[:], rb_ps[:])
                        # attn^T = (P @ V)^T scaled by 1/denom, two 4-bank passes
                        for half in range(2):
                            for d4 in range(4):
                                d_ = half * 4 + d4
                                aps = pspool.tile([128, CH], F32, name=f"at{d4}", tag=f"at{d4}", bufs=1)
                                for k in range(nkt):
                                    nc.tensor.matmul(
                                        aps[:],
                                        vt[k][:, ts(d_, 128)],
                                        pt[k][:],
                                        start=(k == 0), stop=(k == nkt - 1),
                                    )
                                nc.vector.tensor_tensor(
                                    out=attnT[d_][sl // 2][:, ts(sl % 2, CH)],
                                    in0=aps[:], in1=recipB[:],
                                    op=ALU.mult,
                                )

                # ---------------- P3 + P4 ----------------
                with tc.tile_pool(name="p34", bufs=1) as p34:
                    outT = [
                        [p34.tile([128, 512], F16, name=f"oT{dt_}_{qb}", tag=f"oT{dt_}_{qb}") for qb in range(2)]
                        for dt_ in range(NDT)
                    ]
                    # prefetch the first two FFN weight blocks during P3
                    wfb_pool = p34
                    for qb in range(2):
                        for ot in range(NDT):
                            ps = pspool.tile([128, 512], F32, name="mm", tag="mm", bufs=3)
                            for i in range(NDT):
                                nc.tensor.matmul(
                                    ps[:],
                                    wo_sb[i][:, ts(ot, 128)],
                                    attnT[i][qb][:],
                                    start=(i == 0), stop=(i == NDT - 1),
                                )
                            nc.scalar.activation(
                                outT[ot][qb][:], ps[:], AF.Identity, bias=bo2_sb[:, ot : ot + 1]
                            )

                    # ---------------- P4: FFN + GELU ----------------
                    for mb in range(M // 512):
                        wfb = [
                            wfb_pool.tile([128, 512], F16, name=f"wf{i}", tag=f"wf{i}", bufs=2)
                            for i in range(NDT)
                        ]
                        for i in range(NDT):
                            nc.gpsimd.dma_start(wfb[i][:], wfT[ts(i, 128), ts(mb, 512)])
                        for mt in range(4):
                            m = mb * 4 + mt
                            for qb in range(2):
                                ps = pspool.tile([128, 512], F32, name="mm", tag="mm", bufs=3)
                                for i in range(NDT):
                                    nc.tensor.matmul(
                                        ps[:],
                                        wfb[i][:, ts(mt, 128)],
                                        outT[i][qb][:],
                                        start=(i == 0), stop=(i == NDT - 1),
                                    )
                                st = p34.tile([128, 512], F16, name="ffstage", tag="ffstage", bufs=4)
                                nc.scalar.activation(
                                    st[:], ps[:], AF.Gelu, bias=bf_sb[:, m : m + 1]
                                )
                                # store issued by scalar right after its GELU
                                nc.scalar.dma_start(ffT[ts(m, 128), ts(qb, 512)], st[:])

    nc.compile()
    return nc


def _get_program():
    global _PROGRAM
    if _PROGRAM is None:
        _PROGRAM = _build_program()
    return _PROGRAM


def _owned_chunks(core):
    """The four 256-token chunk indices this core owns, in slot order."""
    if core % 2 == 0:
        return (0, 3, 4, 7)
    return (1, 2, 5, 6)


def _make_in_maps(x, Wq, bq, Wk, bk, Wv, bv, Wo, bo, Wf, bf):
    f32, f16 = np.float32, np.float16
    wqT = np.ascontiguousarray(Wq.T, dtype=f16)
    wkT = np.ascontiguousarray(Wk.T, dtype=f16)
    wvT = np.ascontiguousarray(Wv.T, dtype=f16)
    woT = np.ascontiguousarray(Wo.T, dtype=f16)
    wfT = np.ascontiguousarray(Wf.T, dtype=f16)
    bo2 = (Wo.astype(np.float64) @ bv.astype(np.float64) + bo.astype(np.float64))
    bo2 = np.ascontiguousarray(bo2.astype(f32).reshape(D // 128, 128).T)
    bfT = np.ascontiguousarray(bf.reshape(M // 128, 128).T, dtype=f32)
    iota = (
        np.arange(128, dtype=f32)[:, None]
        + 128.0 * np.arange(S // 128, dtype=f32)[None, :]
    )
    shared = {
        "wqT": wqT, "wkT": wkT, "wvT": wvT, "woT": woT, "wfT": wfT,
        "bq": np.ascontiguousarray(bq.reshape(D // 128, 128).T, dtype=f32),
        "bk": np.ascontiguousarray(bk.reshape(D // 128, 128).T, dtype=f32),
        "bo2": bo2,
        "bfT": bfT,
        "iota_kt": np.ascontiguousarray(iota),
    }
    in_maps = []
    for core in range(N_CORES):
        b = core // 2
        chunks = _owned_chunks(core)
        xTb = np.ascontiguousarray(x[b].T, dtype=f16)  # [D, S]
        half = core % 2  # rank within the pair: rank0 owns tokens 0:S/2
        xkT = np.ascontiguousarray(xTb[:, half * (S // 2) : (half + 1) * (S // 2)])
        xoT = np.ascontiguousarray(
            np.concatenate([xTb[:, c * CH : (c + 1) * CH] for c in chunks], axis=1)
        )
        qp = np.concatenate(
            [np.arange(c * CH, (c + 1) * CH) for c in chunks]
        ).astype(f32)[None, :]
        in_maps.append(
            {**shared, "xT": xTb, "xkT": xkT, "xoT": xoT,
             "qpos": np.ascontiguousarray(qp)}
        )
    return in_maps


def _run(inputs, trace=False, trace_cores=None, tmpdir=None):
    import sys

    if "/opt/trn_rl_repo" not in sys.path:
        sys.path.insert(0, "/opt/trn_rl_repo")
    from concourse.bass_utils import run_bass_kernel_spmd

    nc = _get_program()
    in_maps = _make_in_maps(**inputs)
    res = run_bass_kernel_spmd(
        nc, in_maps, list(range(N_CORES)), trace=trace, trace_cores=trace_cores,
        tmpdir=tmpdir,
    )
    out = np.empty((B, S, M), dtype=np.float32)
    for core in range(N_CORES):
        b = core // 2
        chunks = _owned_chunks(core)
        ffT = res.results[core]["ffT"]  # [M, 1024] fp16
        for sl, c in enumerate(chunks):
            out[b, c * CH : (c + 1) * CH] = (
                ffT[:, sl * CH : (sl + 1) * CH].T.astype(np.float32)
            )
    return out, res


def kernel(**inputs):
    out, _ = _run(inputs)
    return out
